# revision 73
# baseline (speedup 1.0000x reference)
"""Trainium2 Bass kernel for AttentionBlock3D (pre-LN MHA + FFN, eval mode).

Contract: kernel(**inputs) takes the FULL unsharded inputs (numpy, keyed as in
setup_inputs()) and returns the FULL output [2, 256, 16, 16, 16] fp32.

Sharding: 8 cores; core i handles batch b = i//4 and a 1024-token block
(i%4) of the 4096 tokens. The token axis is host-rotated per core so one
shared SPMD program computes attention/FFN for "the first 1024 tokens" while
K/V/LN span all 4096 tokens of its batch. Everything on-chip is
channel-major [C, n] which makes every matmul transpose-free.
"""

import numpy as np

C = 256
HEADS = 8
HD = 32
B = 2
N = 4096
NQ = 1024  # tokens owned per core
FF = 1024
EPS = 1e-5
N_CORES = 8

_CACHE = None
last_results = None  # stash of the most recent BassKernelResults (for profiling)

# exp(8u) ≈ (((c0 + c1·u + c2·u²)²)²)² for |8u| ≤ ~2 — used on the DVE so the
# softmax exp splits across ScalarE and VectorE. Any common scale error
# cancels in the softmax normalization.
_EXPC = (1.00009638, 1.00617531, 0.49687977)


def _register_exp_op():
    from concourse import dve_ops
    from concourse.dve_spec import C0, C1, C2, Bin, Spec
    from concourse.dve_uop import AluOp, DveOpSpec
    from concourse.dve_ops import lower as dve_lower

    name = "EXP2_SCH_ANT"
    for o in dve_ops.OPS:
        if o.name == name:
            return o

    def _ref(in0, in1, c0, c1, c2):
        u = in0.astype(np.float32)
        a = (((u * np.float32(c2) + np.float32(c1)) * u) + np.float32(c0)).astype(
            np.float32)
        s = (a * a).astype(np.float32)
        s = (s * s).astype(np.float32)
        return (s * s).astype(np.float32)

    a2 = Bin(AluOp.ADD,
             Bin(AluOp.MULTIPLY,
                 Bin(AluOp.ADD, Bin(AluOp.MULTIPLY, Src0_(), C2), C1), Src0_()),
             C0)
    s1 = Bin(AluOp.MULTIPLY, a2, a2)
    s2 = Bin(AluOp.MULTIPLY, s1, s1)
    body = Bin(AluOp.MULTIPLY, s2, s2)
    spec = Spec(body=body, reference=_ref)

    row = max(dve_ops._SUB_OPCODE_FOR_NAME.values()) + 1
    assert row < 0x20
    dve_ops._SUB_OPCODE_FOR_NAME[name] = row
    shas = {}
    for ver in ("v3", "v4"):
        s = DveOpSpec(name=name, opcode=row, uops=dve_lower(spec, ver=ver),
                      rd1_en=False)
        shas[ver] = s.sha(ver)
    op = dve_ops.DveOp(name, spec, subdim=False, uops_sha=shas)
    dve_ops.OPS.append(op)
    dve_ops.CUSTOM_DVE_SPECS[name] = spec
    return op


def Src0_():
    from concourse.dve_spec import Src0
    return Src0


def _build_program():
    import concourse.bass as bass
    import concourse.mybir as mybir
    import concourse.tile as tile
    from concourse import bacc

    dt = mybir.dt
    AF = mybir.ActivationFunctionType
    OP = mybir.AluOpType

    nc = bacc.Bacc("TRN2", target_bir_lowering=False, debug=False)

    xb_d = nc.dram_tensor("xb", [C, N], dt.float32, kind="ExternalInput")
    wq_d = nc.dram_tensor("wq", [C, C], dt.bfloat16, kind="ExternalInput")
    wk_d = nc.dram_tensor("wk", [C, C], dt.bfloat16, kind="ExternalInput")
    wv_d = nc.dram_tensor("wv", [C, C], dt.bfloat16, kind="ExternalInput")
    wo_d = nc.dram_tensor("wo", [C, C], dt.bfloat16, kind="ExternalInput")
    w1_d = nc.dram_tensor("w1", [C, FF], dt.bfloat16, kind="ExternalInput")
    w2_d = nc.dram_tensor("w2", [FF, C], dt.bfloat16, kind="ExternalInput")
    # packed per-partition biases: cols = bq(2) bk(2) bo(2) b1(8) b2(2)
    bias_d = nc.dram_tensor("biasp", [128, 16], dt.float32, kind="ExternalInput")
    bv_d = nc.dram_tensor("bv", [C], dt.float32, kind="ExternalInput")
    ones_d = nc.dram_tensor("ones", [128, 128], dt.bfloat16, kind="ExternalInput")
    ones32_d = nc.dram_tensor("ones32", [128, 128], dt.float32, kind="ExternalInput")
    out_d = nc.dram_tensor("out", [C, NQ], dt.float32, kind="ExternalOutput")

    f32, bf16 = dt.float32, dt.bfloat16
    # 1/(8*sqrt(HD)) is folded into Wq on the host; ACT exp gets scale=8,
    # the DVE poly op consumes the pre-scaled scores directly.
    SCALE = 8.0
    exp_op = _register_exp_op()

    with tile.TileContext(nc) as tc:
        with (
            tc.tile_pool(name="consts", bufs=1) as consts,
            tc.tile_pool(name="big", bufs=1) as big,
            tc.tile_pool(name="lntmp", bufs=1) as lntmp,
            tc.tile_pool(name="expool", bufs=4) as expool,
            tc.tile_pool(name="small", bufs=2) as small,
            tc.tile_pool(name="attn", bufs=1) as attn,
            tc.tile_pool(name="psA", bufs=2, space="PSUM") as psA,
            tc.tile_pool(name="psB", bufs=2, space="PSUM") as psB,
            tc.tile_pool(name="drp", bufs=2, space="DRAM") as drp,
        ):
            # ---- constants ----
            wq_s = consts.tile([128, 2, C], bf16, tag="wq")
            wk_s = consts.tile([128, 2, C], bf16, tag="wk")
            wv_s = consts.tile([128, 2, C], bf16, tag="wv")
            wo_s = consts.tile([128, 2, C], bf16, tag="wo")
            w1_s = consts.tile([128, 2, FF], bf16, tag="w1")
            w2_s = consts.tile([128, 8, C], bf16, tag="w2")
            for ki in range(2):
                nc.sync.dma_start(wq_s[:, ki, :], wq_d[ki * 128:(ki + 1) * 128, :])
                nc.sync.dma_start(wk_s[:, ki, :], wk_d[ki * 128:(ki + 1) * 128, :])
                nc.sync.dma_start(wv_s[:, ki, :], wv_d[ki * 128:(ki + 1) * 128, :])
                nc.sync.dma_start(wo_s[:, ki, :], wo_d[ki * 128:(ki + 1) * 128, :])
                nc.sync.dma_start(w1_s[:, ki, :], w1_d[ki * 128:(ki + 1) * 128, :])
            for ki in range(8):
                nc.sync.dma_start(w2_s[:, ki, :], w2_d[ki * 128:(ki + 1) * 128, :])
            bias_s = consts.tile([128, 16], f32, tag="bias")
            nc.sync.dma_start(bias_s[:], bias_d[:])
            bvb_s = consts.tile([128, C], f32, tag="bvb")
            _bva = bv_d[:]
            nc.sync.dma_start(
                bvb_s[:],
                bass.AP(tensor=_bva.tensor, offset=_bva.offset,
                        ap=[[0, 128]] + list(_bva.ap)),
            )
            ones_s = consts.tile([128, 128], bf16, tag="ones")
            nc.sync.dma_start(ones_s[:], ones_d[:])
            ones32_s = consts.tile([128, 128], f32, tag="ones32")
            nc.sync.dma_start(ones32_s[:], ones32_d[:])
            eps_s = consts.tile([128, 1], f32, tag="eps")
            nc.vector.memset(eps_s[:], EPS * C * C)
            zero_s = consts.tile([128, 1], f32, tag="zero")
            nc.vector.memset(zero_s[:], 0.0)

            # ---- x load (512-col chunks, both K-tiles per chunk, ascending
            # so LN1 block 0's dependencies land first) ----
            xb_s = big.tile([128, 2, N], f32, tag="xb")
            for c8 in range(8):
                for ki in range(2):
                    nc.sync.dma_start(
                        xb_s[:, ki, c8 * 512:(c8 + 1) * 512],
                        xb_d[ki * 128:(ki + 1) * 128, c8 * 512:(c8 + 1) * 512],
                    )

            y_s = big.tile([128, 2, N], bf16, tag="y")
            kT_s = big.tile([128, 2, N], bf16, tag="kT")
            qT_s = big.tile([128, 2, NQ], bf16, tag="qT")
            # v, token-major, head-grouped with a ones column per head:
            # [128 tokens, 32 token-tiles, 8 heads * 33]
            va_s = big.tile([128, 32, 8 * 33], bf16, tag="va")
            att_s = attn.tile([128, 2, NQ], bf16, tag="att")
            h1_s = big.tile([128, 2, NQ], f32, tag="h1")
            y2_s = big.tile([128, 2, NQ], bf16, tag="y2")
            z_s = big.tile([128, 8, NQ], bf16, tag="z")
            out_s = big.tile([128, 2, NQ], f32, tag="outs")

            def layernorm(src_ap_fn, n_cols, dst, dst_col0, per_block=None):
                """LN over channels (partition dim, 2 K-tiles) of a channel-major
                activation. src_ap_fn(ki, cols) -> fp32 AP [128, len]; writes
                bf16 normalized values into dst[:, ki, dst_col0 + cols]."""
                BLK = 512
                for nb in range(n_cols // BLK):
                    cs = nb * BLK
                    xbf = lntmp.tile([128, 2, BLK], bf16, tag="ln_xbf")
                    x2 = lntmp.tile([128, 2, BLK], bf16, tag="ln_x2")
                    for ki in range(2):
                        eng = nc.vector if ki == 0 else nc.gpsimd
                        eng.tensor_copy(xbf[:, ki, :], src_ap_fn(ki, cs, BLK))
                        nc.scalar.activation(x2[:, ki, :], src_ap_fn(ki, cs, BLK),
                                             AF.Square, bias=zero_s[:])
                    s_ps = psA.tile([128, 2, 512], f32, tag="psA")
                    for ki in range(2):
                        nc.tensor.matmul(
                            s_ps[:, 0, :], ones_s[:], xbf[:, ki, :],
                            start=(ki == 0), stop=(ki == 1),
                        )
                    for ki in range(2):
                        nc.tensor.matmul(
                            s_ps[:, 1, :], ones_s[:], x2[:, ki, :],
                            start=(ki == 0), stop=(ki == 1),
                        )
                    # t = C*s2 - s1^2 ; y = (C*x - s1) * rsqrt(t + C^2 eps)
                    s1sq = lntmp.tile([128, BLK], f32, tag="ln_s1sq")
                    nc.scalar.activation(s1sq[:], s_ps[:, 0, :], AF.Square,
                                         bias=zero_s[:])
                    tv = lntmp.tile([128, BLK], f32, tag="ln_tv")
                    nc.vector.scalar_tensor_tensor(
                        out=tv[:], in0=s_ps[:, 1, :], scalar=float(C),
                        in1=s1sq[:], op0=OP.mult, op1=OP.subtract)
                    sq = lntmp.tile([128, BLK], f32, tag="ln_sq")
                    nc.scalar.activation(sq[:], tv[:], AF.Sqrt, bias=eps_s[:])
                    rstd = lntmp.tile([128, BLK], f32, tag="ln_rstd")
                    nc.vector.reciprocal_approx_fast(out=rstd[:], in_=sq[:])
                    rstd_bf = lntmp.tile([128, BLK], bf16, tag="ln_rstdbf")
                    nc.vector.tensor_copy(rstd_bf[:], rstd[:])
                    for ki in range(2):
                        xc = lntmp.tile([128, BLK], bf16, tag="ln_xc")
                        e2 = nc.vector if ki == 0 else nc.gpsimd
                        nc.vector.scalar_tensor_tensor(
                            out=xc[:], in0=xbf[:, ki, :], scalar=float(C),
                            in1=s_ps[:, 0, :], op0=OP.mult, op1=OP.subtract)
                        e2.tensor_tensor(
                            out=dst[:, ki, dst_col0 + cs:dst_col0 + cs + BLK],
                            in0=xc[:], in1=rstd_bf[:], op=OP.mult)
                    if per_block is not None:
                        per_block(nb)

            # ---- LN1 + projections, interleaved per 512-column block ----
            va_view = va_s.rearrange("p t (h w) -> p t h w", h=8)
            nc.vector.memset(va_view[:, :, :, 32], 1.0)

            def proj_block(nb):
                cs = nb * 512
                kp = psB.tile([128, 2, 512], f32, tag="psB", name=f"kp{nb}")
                for mi in range(2):
                    for ki in range(2):
                        nc.tensor.matmul(
                            kp[:, mi, :], wk_s[:, ki, mi * 128:(mi + 1) * 128],
                            y_s[:, ki, cs:cs + 512],
                            start=(ki == 0), stop=(ki == 1),
                        )
                for mi in range(2):
                    nc.vector.tensor_scalar(
                        out=kT_s[:, mi, cs:cs + 512], in0=kp[:, mi, :],
                        scalar1=bias_s[:, 2 + mi:3 + mi], scalar2=None, op0=OP.add,
                    )
                if nb < 2:
                    qp = psB.tile([128, 2, 512], f32, tag="psB", name=f"qp{nb}")
                    for mi in range(2):
                        for ki in range(2):
                            nc.tensor.matmul(
                                qp[:, mi, :], wq_s[:, ki, mi * 128:(mi + 1) * 128],
                                y_s[:, ki, cs:cs + 512],
                                start=(ki == 0), stop=(ki == 1),
                            )
                    for mi in range(2):
                        nc.vector.tensor_scalar(
                            out=qT_s[:, mi, cs:cs + 512], in0=qp[:, mi, :],
                            scalar1=bias_s[:, 0 + mi:1 + mi], scalar2=None,
                            op0=OP.add,
                        )
                for nt in range(4 * nb, 4 * nb + 4):
                    vp = psB.tile([128, 256], f32, tag="psB", name=f"vp{nt}")
                    for ki in range(2):
                        nc.tensor.matmul(
                            vp[:], y_s[:, ki, nt * 128:(nt + 1) * 128],
                            wv_s[:, ki, :],
                            start=(ki == 0), stop=(ki == 1),
                        )
                    nc.vector.tensor_tensor(
                        out=va_view[:, nt, :, 0:32],
                        in0=vp.rearrange("p (h w) -> p h w", h=8),
                        in1=bvb_s.rearrange("p (h w) -> p h w", h=8),
                        op=OP.add,
                    )

            layernorm(lambda ki, c0, ln: xb_s[:, ki, c0:c0 + ln], N, y_s, 0,
                      per_block=proj_block)

            # ---- attention ----
            # Two independent streams (qb=0, qb=1) interleaved per head pair
            # g: one stream's exp runs on ScalarE while the other's runs on
            # the DVE poly op, and the PE always has the other stream's
            # QK/AV work -> dense PE stream (keeps HAM at full clock).
            def normalize(av, g, q_cs):
                ck = g // 2
                r0 = 64 * (g % 2)
                av_sb = small.tile([33, 2, 512], bf16, tag="avsb")
                nc.scalar.activation(
                    av_sb.rearrange("p a b -> p (a b)"),
                    av.rearrange("p a b -> p (a b)"),
                    AF.Copy, bias=0.0,
                )
                den_bf = small.tile([2, 512], bf16, tag="denbf")
                nc.sync.dma_start(den_bf[:], av_sb[32:33, :, :])
                den = small.tile([2, 512], f32, tag="den")
                nc.vector.tensor_copy(den[:], den_bf[:])
                rec = small.tile([2, 512], f32, tag="rec")
                nc.vector.reciprocal_approx_fast(out=rec[:], in_=den[:])
                rec_bf = small.tile([2, 512], bf16, tag="recbf")
                nc.vector.tensor_copy(rec_bf[:], rec[:])
                rec_d = drp.tile([2, 512], bf16, tag="recd")
                nc.sync.dma_start(rec_d[:], rec_bf[:])
                rbb = small.tile([128, 512], bf16, tag="rbbf")
                for s in range(2):
                    _r = rec_d[s:s + 1, :]
                    nc.sync.dma_start(
                        rbb[r0 + 32 * s:r0 + 32 * s + 32, :],
                        bass.AP(tensor=_r.tensor, offset=_r.offset,
                                ap=[[0, 32]] + list(_r.ap[1:])),
                    )
                araw = small.tile([128, 512], bf16, tag="araw")
                for s in range(2):
                    nc.sync.dma_start(
                        araw[r0 + 32 * s:r0 + 32 * s + 32, :], av_sb[0:32, s, :])
                nc.gpsimd.tensor_tensor(
                    out=att_s[r0:r0 + 64, ck, q_cs:q_cs + 512],
                    in0=araw[r0:r0 + 64, :], in1=rbb[r0:r0 + 64, :], op=OP.mult,
                )

            for qb in range(2):
                q_cs = qb * 512
                for gp in range(2):  # streams s=0,1 -> head pairs g=2gp+s
                    avs = [psB.tile([33, 2, 512], f32, tag="psB",
                                    name=f"av{qb}_{gp}_{s}") for s in range(2)]

                    def do_av(kt, exs):
                        for s in range(2):
                            for hh in range(2):
                                h = 4 * gp + 2 * s + hh
                                nc.tensor.matmul(
                                    avs[s][:, hh, :],
                                    va_view[:, kt, h, :],
                                    exs[s][:, hh, :],
                                    start=(kt == 0), stop=(kt == 31),
                                )

                    pend = None  # (kt, exs) whose AV is deferred one step
                    for kt in range(32):
                        scs = [psA.tile([128, 2, 512], f32, tag="psA",
                                        name=f"sc{qb}_{gp}_{kt}_{s}")
                               for s in range(2)]
                        for s in range(2):
                            for hh in range(2):
                                rr = 64 * s + 32 * hh
                                nc.tensor.matmul(
                                    scs[s][:, hh, :],
                                    kT_s[rr:rr + 32, gp, kt * 128:(kt + 1) * 128],
                                    qT_s[rr:rr + 32, gp, q_cs:q_cs + 512],
                                    start=True, stop=True,
                                    tile_position=(rr, 0),
                                )
                        exs = []
                        for s in range(2):
                            ex = expool.tile([128, 2, 512], bf16, tag="ex")
                            exs.append(ex)
                            if (2 * kt + s) % 8 in (1, 4, 6):
                                nc.vector._custom_dve(
                                    exp_op,
                                    out=ex.rearrange("p a b -> p (a b)"),
                                    in0=scs[s].rearrange("p a b -> p (a b)"),
                                    s0=_EXPC[0], s1=_EXPC[1], imm2=_EXPC[2],
                                )
                            else:
                                nc.scalar.activation(
                                    ex.rearrange("p a b -> p (a b)"),
                                    scs[s].rearrange("p a b -> p (a b)"),
                                    AF.Exp, scale=SCALE,
                                )
                        if pend is not None:
                            do_av(*pend)
                        pend = (kt, exs)
                    do_av(*pend)
                    for s in range(2):
                        normalize(avs[s], 2 * gp + s, q_cs)

            # ---- out projection + residual ----
            for qb in range(2):
                q_cs = qb * 512
                for mi in range(2):
                    op_ps = psB.tile([128, 512], f32, tag="psB")
                    for ck in range(2):
                        nc.tensor.matmul(
                            op_ps[:], wo_s[:, ck, mi * 128:(mi + 1) * 128],
                            att_s[:, ck, q_cs:q_cs + 512],
                            start=(ck == 0), stop=(ck == 1),
                        )
                    nc.vector.scalar_tensor_tensor(
                        out=h1_s[:, mi, q_cs:q_cs + 512],
                        in0=op_ps[:], scalar=bias_s[:, 4 + mi:5 + mi],
                        in1=xb_s[:, mi, q_cs:q_cs + 512],
                        op0=OP.add, op1=OP.add,
                    )

            # ---- LN2 + FFN, interleaved per 512-token block ----
            def ffn_block(qb):
                q_cs = qb * 512
                for mi8 in range(8):
                    zp = psA.tile([128, 512], f32, tag="psA", name=f"zp{qb}_{mi8}")
                    for ki in range(2):
                        nc.tensor.matmul(
                            zp[:], w1_s[:, ki, mi8 * 128:(mi8 + 1) * 128],
                            y2_s[:, ki, q_cs:q_cs + 512],
                            start=(ki == 0), stop=(ki == 1),
                        )
                    nc.scalar.activation(
                        z_s[:, mi8, q_cs:q_cs + 512], zp[:], AF.Relu,
                        bias=bias_s[:, 6 + mi8:7 + mi8],
                    )
                for mi in range(2):
                    fp = psB.tile([128, 512], f32, tag="psB", name=f"fp{qb}_{mi}")
                    for ki8 in range(8):
                        nc.tensor.matmul(
                            fp[:], w2_s[:, ki8, mi * 128:(mi + 1) * 128],
                            z_s[:, ki8, q_cs:q_cs + 512],
                            start=(ki8 == 0), stop=(ki8 == 7),
                        )
                    nc.vector.scalar_tensor_tensor(
                        out=out_s[:, mi, q_cs:q_cs + 512],
                        in0=fp[:], scalar=bias_s[:, 14 + mi:15 + mi],
                        in1=h1_s[:, mi, q_cs:q_cs + 512],
                        op0=OP.add, op1=OP.add,
                    )
                    nc.sync.dma_start(
                        out_d[mi * 128:(mi + 1) * 128, q_cs:q_cs + 512],
                        out_s[:, mi, q_cs:q_cs + 512],
                    )

            layernorm(lambda ki, c0, ln: h1_s[:, ki, c0:c0 + ln], NQ, y2_s, 0,
                      per_block=ffn_block)

    nc.compile()
    return nc


def _prepare_in_maps(inputs):
    import ml_dtypes

    bf16 = ml_dtypes.bfloat16
    f32 = np.float32
    x = np.ascontiguousarray(inputs["x"], dtype=f32).reshape(B, C, N)
    wq, wk, wv, wo = (np.asarray(inputs[k], f32) for k in ("wq", "wk", "wv", "wo"))
    bq, bk, bv, bo = (np.asarray(inputs[k], f32) for k in ("bq", "bk", "bv", "bo"))
    g1, b1, g2, b2 = (np.asarray(inputs[k], f32) for k in ("g1", "b1", "g2", "b2"))
    w1, bf1, w2, bf2 = (np.asarray(inputs[k], f32) for k in ("w1", "bf1", "w2", "bf2"))

    qscale = np.float32(1.0 / (8.0 * np.sqrt(HD)))
    wq_e = (wq * g1[:, None] * qscale).astype(bf16)
    wk_e = (wk * g1[:, None]).astype(bf16)
    wv_e = (wv * g1[:, None]).astype(bf16)
    wo_e = wo.astype(bf16)
    w1_e = (w1 * g2[:, None]).astype(bf16)
    w2_e = w2.astype(bf16)
    bq_e = (bq + b1 @ wq) * qscale
    bk_e = bk + b1 @ wk
    bv_e = bv + b1 @ wv
    bo_e = bo
    b1_e = bf1 + b2 @ w1
    b2_e = bf2

    biasp = np.zeros((128, 16), f32)
    biasp[:, 0] = bq_e[0:128]
    biasp[:, 1] = bq_e[128:256]
    biasp[:, 2] = bk_e[0:128]
    biasp[:, 3] = bk_e[128:256]
    biasp[:, 4] = bo_e[0:128]
    biasp[:, 5] = bo_e[128:256]
    for j in range(8):
        biasp[:, 6 + j] = b1_e[j * 128:(j + 1) * 128]
    biasp[:, 14] = b2_e[0:128]
    biasp[:, 15] = b2_e[128:256]

    ones = np.ones((128, 128), bf16)

    shared = {
        "wq": wq_e, "wk": wk_e, "wv": wv_e, "wo": wo_e,
        "w1": w1_e, "w2": w2_e, "biasp": biasp, "bv": bv_e,
        "ones": ones, "ones32": np.ones((128, 128), f32),
    }
    in_maps = []
    for i in range(N_CORES):
        b = i // 4
        q0 = (i % 4) * NQ
        xb_rot = np.ascontiguousarray(np.roll(x[b], -q0, axis=1))
        m = dict(shared)
        m["xb"] = xb_rot
        in_maps.append(m)
    return in_maps


def kernel(**inputs):
    global _CACHE, last_results
    from concourse.bass_utils import run_bass_kernel_spmd

    if _CACHE is None:
        _CACHE = _build_program()
    nc = _CACHE
    in_maps = _prepare_in_maps(inputs)
    res = run_bass_kernel_spmd(nc, in_maps, core_ids=list(range(N_CORES)))
    last_results = res
    full = np.zeros((B, C, N), np.float32)
    for i in range(N_CORES):
        b = i // 4
        q0 = (i % 4) * NQ
        full[b][:, q0:q0 + NQ] = res.results[i]["out"]
    return full.reshape(B, C, 16, 16, 16)
